# revision 11
# baseline (speedup 1.0000x reference)
"""BeamSplitterAttention on 8 TRN2 NeuronCores (Bass/Tile, SPMD).

Decomposition (v2, bf16):
  Phase A (local stage: nc=4 chunk-wise attention): sharded over the
    chunk-position axis. Each core gets 512 rows of x (64 positions x 2
    batches x 4 chunks) pre-transposed as xaT [d, r]; computes qkv on the
    PE (bf16), the tiny nc=4 softmax-attention on the Vector engine, and
    outputs the TRANSPOSED attention output oT (the local out-projection
    is folded into phase B's qkv weights on the host:
    W_comb = w_in_g @ w_out_l, b_comb = b_in_g + w_in_g @ b_out_l).
  Phase B (global stage: full attention over S=2048): tensor-parallel over
    the H=16 heads (2 per core). Each core consumes the full oT, its 2
    heads' combined-qkv weight slices, runs scores (PE, bf16, K=64
    alternating row-quadrants) -> exp (ACT, fused 1/sqrt(64) scale, no
    max-subtraction needed at these scales) -> attn@V (PE, bf16; a
    ones-column in the V tiles yields the softmax denominators in psum
    row 64), then a partial out-projection over its 128 concat features.
    Host sums the 8 partial outputs and adds b_out_g.

All matmul operands are bf16 (psum accumulation is fp32); softmax
statistics are fp32. Each block's normalize+out-proj tail is emitted
inside the next block's score stream so the in-order PE queue never
stalls on the ACT/DVE chain.
"""
import numpy as np
import ml_dtypes

import concourse.bass as bass
import concourse.tile as tile
from concourse import bacc, mybir
from concourse.bass_utils import run_bass_kernel_spmd
from concourse.masks import make_identity

F32 = mybir.dt.float32
WDT = mybir.dt.bfloat16
NPW = ml_dtypes.bfloat16
AX = mybir.AxisListType
OP = mybir.AluOpType
ACTF = mybir.ActivationFunctionType
HINTS = (mybir.EngineType.PE, mybir.EngineType.Activation,
         mybir.EngineType.DVE, mybir.EngineType.SP)

P = 128
D = 1024
H = 16
HD = 64
B = 2
S = 2048
NCH = 4
NCORES = 8
ROWS_A = 512
RT_B = 8

ABLATE = set()


def _loop(tc, loop_r):
    import contextlib
    if loop_r:
        return tc.For_i(0, loop_r, 1, hint_engines=HINTS)
    return contextlib.nullcontext()


# ---------------------------- phase A ----------------------------

def build_phase_a(loop_r=None):
    nc = bacc.Bacc("TRN2", target_bir_lowering=False, debug=False,
                   num_devices=NCORES)
    xaT = nc.dram_tensor("xaT", (D, ROWS_A), WDT, kind="ExternalInput").ap()
    winlT = nc.dram_tensor("winlT", (D, 3 * D), WDT, kind="ExternalInput").ap()
    binl = nc.dram_tensor("binl", (3 * D,), WDT, kind="ExternalInput").ap()
    oTa = nc.dram_tensor("oTa", (D, ROWS_A), WDT, kind="ExternalOutput").ap()

    with tile.TileContext(nc) as tc:
        with nc.allow_low_precision(reason="bf16 matmul operands by design"):
            _phase_a_body(tc, xaT, winlT, binl, oTa, loop_r)
    nc.compile()
    return nc


def _phase_a_body(tc, xaT, winlT, binl, oTa, loop_r=None):
    nc = tc.nc
    KT = 8

    import contextlib
    ctx = contextlib.ExitStack()
    const = ctx.enter_context(tc.tile_pool(name="const", bufs=1))
    wpool = ctx.enter_context(tc.tile_pool(name="w", bufs=2))
    tpool = ctx.enter_context(tc.tile_pool(name="tmp", bufs=2))
    apool = ctx.enter_context(tc.tile_pool(name="attn", bufs=1))
    psq = ctx.enter_context(tc.tile_pool(name="psq", bufs=6, space="PSUM"))
    pst = ctx.enter_context(tc.tile_pool(name="pst", bufs=2, space="PSUM"))

    xa = const.tile([P, KT, ROWS_A], WDT)
    nc.sync.dma_start(xa[:], xaT.rearrange("(kt p) r -> p kt r", p=P))
    bias_sb = const.tile([1, 3 * D], WDT)
    nc.sync.dma_start(bias_sb[:], binl.rearrange("(a f) -> a f", a=1))
    ones_r = const.tile([1, ROWS_A], WDT)
    nc.vector.memset(ones_r[:], 1.0)
    ident = const.tile([P, P], WDT)
    make_identity(nc, ident)

    qkv = const.tile([P, NCH, 3 * D], WDT)
    o_sb = const.tile([P, NCH, D], WDT)
    oT = const.tile([P, KT, ROWS_A], WDT)

    with _loop(tc, loop_r):
        _phase_a_compute(tc, oTa, winlT, xa, bias_sb, ones_r, ident,
                         qkv, o_sb, oT, wpool, tpool, apool, psq, pst)
    ctx.close()


def _phase_a_compute(tc, oTa, winlT, xa, bias_sb, ones_r, ident,
                     qkv, o_sb, oT, wpool, tpool, apool, psq, pst):
    nc = tc.nc
    KT = 8
    H2 = 8  # heads per half

    for hh in range(2):
        # qkv projection for this head half (q, k, v feature tiles)
        for sec in range(3):
            ft = 2 * sec + hh
            wts = wpool.tile([P, KT, 512], WDT, tag="winl", name=f"w_{hh}_{sec}")
            nc.sync.dma_start(
                wts[:],
                winlT[:, ft * 512:(ft + 1) * 512].rearrange("(kt p) f -> p kt f", p=P),
            )
            for c in range(NCH):
                ps = psq.tile([P, 512], F32, tag="mm", name=f"ps_{hh}_{sec}_{c}")
                for kt in range(KT):
                    nc.tensor.matmul(ps[:], xa[:, kt, c * P:(c + 1) * P],
                                     wts[:, kt, :], start=(kt == 0), stop=False)
                nc.tensor.matmul(ps[:], ones_r[0:1, 0:P],
                                 bias_sb[0:1, ft * 512:(ft + 1) * 512],
                                 start=False, stop=True)
                nc.scalar.copy(out=qkv[:, c, ft * 512:(ft + 1) * 512], in_=ps[:])

        # local attention over chunks (nc=4) on DVE, this half
        qv = qkv[:, :, 512 * hh: 512 * hh + 512].rearrange(
            "p c (h d) -> p c h d", h=H2)
        kv = qkv[:, :, D + 512 * hh: D + 512 * hh + 512].rearrange(
            "p c (h d) -> p c h d", h=H2)
        vv = qkv[:, :, 2 * D + 512 * hh: 2 * D + 512 * hh + 512].rearrange(
            "p c (h d) -> p c h d", h=H2)

        s_t = apool.tile([P, NCH, H2, NCH], F32, tag="s", name=f"s_{hh}")
        for ck in range(NCH):
            tmp = tpool.tile([P, NCH, H2, HD], F32, tag="tmp", name=f"tmp_{hh}_{ck}")
            kb = kv[:, ck][:, None].to_broadcast((P, NCH, H2, HD))
            nc.vector.tensor_tensor(tmp[:], qv[:], kb, OP.mult)
            nc.vector.reduce_sum(s_t[:, :, :, ck], tmp[:], axis=AX.X)

        e_t = apool.tile([P, NCH, H2, NCH], F32, tag="e", name=f"e_{hh}")
        nc.scalar.activation(e_t[:], s_t[:], ACTF.Exp, scale=1.0 / np.sqrt(HD))
        den = apool.tile([P, NCH, H2], F32, tag="den", name=f"den_{hh}")
        nc.vector.reduce_sum(den[:], e_t[:], axis=AX.X)
        nc.vector.reciprocal(den[:], den[:])
        nc.vector.tensor_tensor(
            e_t[:], e_t[:],
            den[:, :, :, None].to_broadcast((P, NCH, H2, NCH)), OP.mult)

        ov = o_sb[:, :, 512 * hh: 512 * hh + 512].rearrange(
            "p c (h d) -> p c h d", h=H2)
        for ck in range(NCH):
            eb = e_t[:, :, :, ck][:, :, :, None].to_broadcast((P, NCH, H2, HD))
            vb = vv[:, ck][:, None].to_broadcast((P, NCH, H2, HD))
            if ck == 0:
                nc.vector.tensor_tensor(ov[:], eb, vb, OP.mult)
            else:
                tmp = tpool.tile([P, NCH, H2, HD], F32, tag="tmp", name=f"tmpo_{hh}_{ck}")
                nc.vector.tensor_tensor(tmp[:], eb, vb, OP.mult)
                nc.vector.tensor_tensor(ov[:], ov[:], tmp[:], OP.add)

        # transpose this half of o -> oT (bf16 PE transpose)
        for cq in range(NCH):
            for ftl in range(4 * hh, 4 * hh + 4):
                tps = pst.tile([P, P], WDT, tag="tp", name=f"tp_{hh}_{cq}_{ftl}")
                nc.tensor.transpose(tps[:], o_sb[:, cq, ftl * P:(ftl + 1) * P],
                                    ident[:])
                nc.scalar.copy(out=oT[:, ftl, cq * P:(cq + 1) * P], in_=tps[:])

    for dt in range(KT):
        nc.sync.dma_start(oTa[dt * P:(dt + 1) * P, :], oT[:, dt, :])


# ---------------------------- phase B ----------------------------

def build_phase_b(loop_r=None):
    nc = bacc.Bacc("TRN2", target_bir_lowering=False, debug=False,
                   num_devices=NCORES)
    locT = nc.dram_tensor("locT", (D, B * S), WDT, kind="ExternalInput").ap()
    wqT = nc.dram_tensor("wqT", (D, P), WDT, kind="ExternalInput").ap()
    wkT = nc.dram_tensor("wkT", (D, P), WDT, kind="ExternalInput").ap()
    wvT = nc.dram_tensor("wvT", (D, P), WDT, kind="ExternalInput").ap()
    bq = nc.dram_tensor("bq", (P,), F32, kind="ExternalInput").ap()
    bk = nc.dram_tensor("bk", (P,), F32, kind="ExternalInput").ap()
    bv = nc.dram_tensor("bv", (P,), F32, kind="ExternalInput").ap()
    woT = nc.dram_tensor("woT", (P, D), WDT, kind="ExternalInput").ap()
    outTp = nc.dram_tensor("outTp", (D, B * S), F32, kind="ExternalOutput").ap()

    with tile.TileContext(nc) as tc:
        with nc.allow_low_precision(reason="bf16 matmul operands by design"):
            _phase_b_body(tc, locT, wqT, wkT, wvT, bq, bk, bv, woT, outTp, loop_r)
    nc.compile()
    return nc


def _phase_b_body(tc, locT, wqT, wkT, wvT, bq, bk, bv, woT, outTp, loop_r=None):
    nc = tc.nc
    KT = 8

    import contextlib
    ctx = contextlib.ExitStack()
    const = ctx.enter_context(tc.tile_pool(name="const", bufs=1))
    lpool = ctx.enter_context(tc.tile_pool(name="loc", bufs=3))
    epool = ctx.enter_context(tc.tile_pool(name="exp", bufs=6))
    apool = ctx.enter_context(tc.tile_pool(name="attn", bufs=2))
    dpool = ctx.enter_context(tc.tile_pool(name="den", bufs=2))
    outp = ctx.enter_context(tc.tile_pool(name="out", bufs=3))
    vt_pool = ctx.enter_context(tc.tile_pool(name="vt", bufs=2))
    ps_big = ctx.enter_context(tc.tile_pool(name="ps_big", bufs=2, space="PSUM"))
    ps_st1 = ctx.enter_context(tc.tile_pool(name="ps_st1", bufs=1, space="PSUM"))
    ps_opj = ctx.enter_context(tc.tile_pool(name="ps_opj", bufs=1, space="PSUM"))
    ps_acc = ctx.enter_context(tc.tile_pool(name="ps_acc", bufs=2, space="PSUM"))

    wq_t = const.tile([P, KT, P], WDT)
    nc.sync.dma_start(wq_t[:], wqT.rearrange("(kt p) f -> p kt f", p=P))
    wk_t = const.tile([P, KT, P], WDT)
    nc.sync.dma_start(wk_t[:], wkT.rearrange("(kt p) f -> p kt f", p=P))
    wv_t = const.tile([P, KT, P], WDT)
    nc.sync.dma_start(wv_t[:], wvT.rearrange("(kt p) f -> p kt f", p=P))
    wo_t = const.tile([P, D], WDT)
    nc.sync.dma_start(wo_t[:], woT[:, :])
    bq_t = const.tile([P, 1], F32)
    nc.sync.dma_start(bq_t[:], bq.rearrange("(o p) -> p o", p=P))
    bk_t = const.tile([P, 1], F32)
    nc.sync.dma_start(bk_t[:], bk.rearrange("(o p) -> p o", p=P))
    bv_t = const.tile([P, 1], F32)
    nc.sync.dma_start(bv_t[:], bv.rearrange("(o p) -> p o", p=P))
    ident = const.tile([P, P], WDT)
    make_identity(nc, ident)
    ones_f = const.tile([P, HD], WDT)
    nc.vector.memset(ones_f[:], 1.0)

    qT = const.tile([P, B * S], mybir.dt.float32r)
    kT = const.tile([P, B * S], mybir.dt.float32r)
    v_sb = const.tile([P, 32, 130], WDT)
    nc.vector.tensor_copy(
        v_sb.rearrange("p r (j f) -> p r j f", f=65)[:, :, :, 64],
        ones_f.rearrange("p (a b) -> p a b", b=2),
    )

    with _loop(tc, loop_r):
        _phase_b_compute(tc, locT, outTp, wq_t, wk_t, wv_t, wo_t,
                         bq_t, bk_t, bv_t, ident,
                         qT, kT, v_sb, lpool, epool, apool, dpool, outp,
                         vt_pool, ps_big, ps_st1, ps_opj, ps_acc)
    ctx.close()


def _phase_b_compute(tc, locT, outTp, wq_t, wk_t, wv_t, wo_t,
                     bq_t, bk_t, bv_t, ident,
                     qT, kT, v_sb, lpool, epool, apool, dpool, outp,
                     vt_pool, ps_big, ps_st1, ps_opj, ps_acc):
    nc = tc.nc
    KT = 8
    NKT = 16
    NQT = 4
    LAG = 3

    # stage 1 (q/k/v projections + v transpose) is cut into small emission
    # pieces that get interleaved into the attention instruction stream, so
    # its PE work fills the PE idle gaps of the ACT-bound attention loop.
    def make_rt_pieces(rt):
        rsl = slice(rt * 512, (rt + 1) * 512)
        st = {}

        def dma():
            loc_t = lpool.tile([P, KT, 512], WDT, tag="loc", name=f"loc{rt}")
            nc.sync.dma_start(loc_t[:],
                              locT[:, rsl].rearrange("(kt p) r -> p kt r", p=P))
            st["loc"] = loc_t

        def mm(key, w_t, i0):
            def f():
                if i0 == 0:
                    st[key] = ps_st1.tile([P, 512], F32, tag="s",
                                          name=f"ps_{key}{rt}")
                for kt in (i0, i0 + 1):
                    nc.tensor.matmul(st[key][:], w_t[:, kt, :],
                                     st["loc"][:, kt, :],
                                     start=(kt == 0), stop=(kt == KT - 1))
            return f

        kp = [mm("k", wk_t, i) for i in (0, 2, 4, 6)]

        def fin_k():
            nc.vector.tensor_scalar_add(kT[:, rsl], st["k"][:], bk_t[:, 0:1])
        kp.append(fin_k)

        vp = [mm("v", wv_t, i) for i in (0, 2, 4, 6)]

        def fin_v():
            vt = vt_pool.tile([P, 512], WDT, tag="vts", name=f"vt{rt}")
            nc.vector.tensor_copy(vt[:], st["v"][:])
            st["vt"] = vt
        vp.append(fin_v)

        def tr(i):
            def f():
                tps = ps_st1.tile([P, P], WDT, tag="s", name=f"tp{rt}_{i}")
                nc.tensor.transpose(tps[:], st["vt"][:, i * P:(i + 1) * P],
                                    ident[:])
                nc.vector.tensor_copy(
                    v_sb[:, rt * 4 + i, :].rearrange("p (j f) -> p j f",
                                                     j=2)[:, :, 0:64],
                    tps[:].rearrange("p (j f) -> p j f", j=2))
            return f
        vp += [tr(i) for i in range(4)]

        qp = [mm("q", wq_t, i) for i in (0, 2, 4, 6)]

        def fin_q():
            nc.vector.tensor_scalar_add(qT[:, rsl], st["q"][:], bq_t[:, 0:1])
        qp.append(fin_q)

        return {"dma": [dma], "k": kp, "v": vp, "q": qp}

    all_rt = {rt: make_rt_pieces(rt) for rt in range(RT_B)}

    def flat(rts):
        out = []
        for key in ("dma", "k", "v", "q"):
            for rt in rts:
                out += all_rt[rt][key]
        return out

    if "stage1_only" in ABLATE:
        for f in flat(list(range(RT_B))):
            f()
        return

    # serial prefix: rows consumed immediately by the first attention block
    for f in flat([0, 1]):
        f()
    # remaining rows hide inside attention blocks; keys ordered so each rt's
    # kT lands before its scores deadline and v before its attn@V deadline.
    SCHED = {(0, 0): [2, 3], (0, 1): [4], (0, 2): [5], (0, 3): [6], (1, 0): [7]}

    # stage 2: attention, with each block's normalize+out-proj tail emitted
    # piecewise into the NEXT block's score stream (one piece per kt slot)
    # so the in-order PE/DVE queues never stall on the tail chain.
    def make_tail(o_ps, attn, qsl):
        pieces = []

        def norm(j):
            def f():
                den = dpool.tile([1, 512], F32, tag="den")
                nc.vector.reciprocal(den[0:1, :], o_ps[j][64:65, :])
                bc_sb = dpool.tile([64, 512], F32, tag="bcsb")
                nc.gpsimd.partition_broadcast(bc_sb[:], den[0:1, :])
                nc.vector.tensor_tensor(attn[64 * j:64 * j + 64, :],
                                        o_ps[j][0:64, :], bc_sb[:], OP.mult)
                nc.vector.tensor_scalar_add(attn[64 * j:64 * j + 64, :],
                                            attn[64 * j:64 * j + 64, :],
                                            bv_t[64 * j:64 * j + 64, 0:1])
            return f

        def oproj(dt):
            def f():
                ps = ps_opj.tile([P, 512], F32, tag="m")
                nc.tensor.matmul(ps[:], wo_t[:, dt * P:(dt + 1) * P], attn[:],
                                 start=True, stop=True)
                ot = outp.tile([P, 512], F32, tag="out")
                nc.vector.tensor_copy(ot[:], ps[:])
                nc.sync.dma_start(outTp[dt * P:(dt + 1) * P, qsl], ot[:])
            return f

        pieces.append(norm(0))
        pieces.append(norm(1))
        for dt in range(KT):
            pieces.append(oproj(dt))
        return pieces

    pending = []
    for b in range(B):
        for qt in range(NQT):
            qsl = slice(b * S + qt * 512, b * S + (qt + 1) * 512)
            attn = apool.tile([P, 512], WDT, tag="attn", name=f"attn{b}_{qt}")
            s1q = flat(SCHED.get((b, qt), []))
            o_ps = None
            e_ts = {}

            def do_av(kt, b=b, qt=qt, e_ts=e_ts):
                e_kt = e_ts.pop(kt)
                for j in range(2):
                    nc.tensor.matmul(o_ps[j][:],
                                     v_sb[:, b * 16 + kt, 65 * j:65 * j + 65],
                                     e_kt[:, j * 512:(j + 1) * 512],
                                     start=(kt == 0), stop=(kt == NKT - 1))

            for kt in range(NKT):
                ksl = slice(b * S + kt * P, b * S + (kt + 1) * P)
                sps = ps_big.tile([P, 1024], F32, tag="sps")
                e_t = epool.tile([P, 1024], WDT, tag="et", name=f"e_t{b}_{qt}_{kt}")
                for j in range(2):
                    fsl = slice(64 * j, 64 * j + 64)
                    nc.tensor.matmul(sps[:, j * 512:(j + 1) * 512],
                                     kT[fsl, ksl], qT[fsl, qsl],
                                     start=True, stop=True)
                if "no_exp" not in ABLATE:
                    nc.scalar.activation(e_t[:], sps[:], ACTF.Exp, scale=1.0 / np.sqrt(HD))
                else:
                    nc.vector.tensor_copy(e_t[:, 0:8], sps[:, 0:8])
                e_ts[kt] = e_t
                if pending and kt >= 1:
                    pending.pop(0)()
                if s1q:
                    quota = -(-len(s1q) // max(1, 14 - kt))
                    for _ in range(min(quota, 3, len(s1q))):
                        s1q.pop(0)()
                if kt == LAG - 1:
                    o_ps = [ps_acc.tile([65, 512], F32, tag="o",
                                        name=f"o_ps{b}_{qt}_{j}") for j in range(2)]
                if kt >= LAG:
                    do_av(kt - LAG)
            for kt in range(NKT - LAG, NKT):
                do_av(kt)
            assert not s1q

            if "no_tail" in ABLATE:
                continue
            assert not pending
            pending = make_tail(o_ps, attn, qsl)
    for f in pending:
        f()


# ---------------- host-side prep / assembly ----------------

def _bf(x):
    return np.ascontiguousarray(np.asarray(x, np.float32).astype(NPW))


def prep_phase_a_inputs(x, w_in_l, b_in_l):
    x = np.asarray(x, np.float32)
    xr = x.reshape(B, NCH, NCORES, 64, D)
    xa = np.transpose(xr, (2, 1, 0, 3, 4)).reshape(NCORES, ROWS_A, D)
    winlT = _bf(np.asarray(w_in_l, np.float32).T)
    binl = _bf(b_in_l)
    in_maps = []
    for k in range(NCORES):
        in_maps.append({
            "xaT": _bf(xa[k].T),
            "winlT": winlT,
            "binl": binl,
        })
    return in_maps


def assemble_locT(results):
    A = np.stack([np.asarray(results[k]["oTa"]) for k in range(NCORES)])
    locT = (A.reshape(NCORES, D, NCH, B, 64)
             .transpose(1, 2, 3, 0, 4)
             .reshape(D, B * S))
    return np.ascontiguousarray(locT)


def prep_phase_b_inputs(locT, w_in_g, b_in_g, w_out_g):
    w_in_g = np.asarray(w_in_g, np.float32)
    b_in_g = np.asarray(b_in_g, np.float32)
    w_out_g = np.asarray(w_out_g, np.float32)
    in_maps = []
    for k in range(NCORES):
        sl = slice(128 * k, 128 * k + 128)
        in_maps.append({
            "locT": locT,
            "wqT": _bf(w_in_g[sl, :].T),
            "wkT": _bf(w_in_g[D + 128 * k: D + 128 * k + 128, :].T),
            "wvT": _bf(w_in_g[2 * D + 128 * k: 2 * D + 128 * k + 128, :].T),
            "bq": np.ascontiguousarray(b_in_g[sl]),
            "bk": np.ascontiguousarray(b_in_g[D + 128 * k: D + 128 * k + 128]),
            "bv": np.ascontiguousarray(b_in_g[2 * D + 128 * k: 2 * D + 128 * k + 128]),
            "woT": _bf(w_out_g[:, sl].T),
        })
    return in_maps


def assemble_output(results, b_out_g):
    outT = np.sum([np.asarray(results[k]["outTp"]) for k in range(NCORES)], axis=0)
    outT += np.asarray(b_out_g, np.float32)[:, None]
    return np.ascontiguousarray(outT.T.reshape(B, S, D))


_CACHE = {}


def kernel(x, w_in_l, b_in_l, w_out_l, b_out_l, w_in_g, b_in_g, w_out_g, b_out_g):
    if "a" not in _CACHE:
        _CACHE["a"] = build_phase_a()
    if "b" not in _CACHE:
        _CACHE["b"] = build_phase_b()
    core_ids = list(range(NCORES))
    # Fold the local out-projection into the global qkv projection:
    # loc = o @ w_out_l.T + b_out_l  =>  qkv_g = o @ (w_in_g @ w_out_l).T
    #                                           + (b_in_g + w_in_g @ b_out_l)
    w_in_l = np.asarray(w_in_l, np.float32)
    w_in_g = np.asarray(w_in_g, np.float32)
    w_out_l = np.asarray(w_out_l, np.float32)
    w_comb = w_in_g @ w_out_l
    b_comb = (np.asarray(b_in_g, np.float32)
              + w_in_g @ np.asarray(b_out_l, np.float32))
    in_a = prep_phase_a_inputs(x, w_in_l, b_in_l)
    res_a = run_bass_kernel_spmd(_CACHE["a"], in_a, core_ids=core_ids)
    oT = assemble_locT(res_a.results)
    in_b = prep_phase_b_inputs(oT, w_comb, b_comb, w_out_g)
    res_b = run_bass_kernel_spmd(_CACHE["b"], in_b, core_ids=core_ids)
    return assemble_output(res_b.results, b_out_g)
